# revision 10
# baseline (speedup 1.0000x reference)
"""GAT layer on 8 TRN2 cores: dst-sharded, edge-gather + one-hot segment matmul.

Design (v8, ~0.81 ms HW exec vs 1.77-2.01 ms baseline):
  - Output 128-dst-node windows are load-balanced across the 8 cores by edge
    count (host permutes window->core assignment, reassembles the output).
  - Phase 1b (first; table-independent): a_dst logits for assigned windows.
  - Phase 1: full transformed-feature table h||a_src||a_dst = x @ [W | W@att]
    (one 264-col matmul per 128-node tile, PSUM->SBUF copies alternating
    scalar/vector, row pad pre-zeroed per buffer, 8 tiles per DRAM write).
    lo/hi fences emitted inline: lo-half gathers start at ~50% of phase 1.
  - Phase 2, ragged per-window schedule: per (window, stream) slot count =
    MAX edge count over cores (pad with index 0 to that count, -1 beyond);
    SPMD-uniform num_idxs_reg == count keeps the gather ucode's trailing-
    negative trim consistent with the sequencer's ring accounting (any
    mismatch wedges the device).  Gathers cycle SWDGE queues 0-3 (per-queue
    Q7 core pairs, ~2 in flight).  Gather buffers pre-zeroed up front.
    Scores exp(leakyrelu(a_src + a_dst)) via one-hot a_dst matmuls, batched
    per-stream msg multiply, segment-sum into PSUM via one-hot matmuls with
    denominators riding in cols 256:260.  The per-window normalize (scalar-
    engine per-partition reciprocal multiply) is DEFERRED two windows so its
    queue entries never stall the next windows' score ops.
"""
import sys
sys.path.insert(0, '/opt/trn_rl_repo')
import numpy as np
import ml_dtypes

import bass_rust as _br
import concourse.bacc as bacc
import concourse.mybir as mybir
import concourse.tile as tile
from concourse import bass_utils

BF16 = ml_dtypes.bfloat16
FP8 = ml_dtypes.float8_e4m3

C_IN = 128
C_OUT_TOT = 256   # HEADS * OUT_CH
HEADS = 4
HC = 64
NEG_SLOPE = 0.2
ROW = 384         # table row: 256 h + 4 a_src + 4 a_dst + 120 pad (bf16) = 768 B
PREF = 8          # lo-gather prefetch depth (windows)


def host_prep(x, edge_index, W, att_src, att_dst, bias, n_cores=8):
    """Shard + schedule. Returns (cfg, in_maps, assign)."""
    N = x.shape[0]
    src = np.concatenate([np.asarray(edge_index[0], np.int64),
                          np.arange(N, dtype=np.int64)]).astype(np.int32)
    dst = np.concatenate([np.asarray(edge_index[1], np.int64),
                          np.arange(N, dtype=np.int64)]).astype(np.int32)

    NPC = N // n_cores
    NW = (NPC + 127) // 128
    SPLIT = (N + 1) // 2
    assert SPLIT < 32768 and (N - SPLIT) < 32768

    # global 128-dst windows within each core's contiguous range
    wins = []          # (lo, hi) global dst range per window
    for c in range(n_cores):
        for w in range(NW):
            lo = c * NPC + w * 128
            hi = min(c * NPC + (w + 1) * 128, (c + 1) * NPC)
            wins.append((lo, hi))
    NGW = len(wins)    # n_cores * NW

    order = np.argsort(dst, kind='stable')
    src_s, dst_s = src[order], dst[order]
    wlists = []        # per window: [(src, dst_local) per stream]
    wcount = np.zeros(NGW, np.int64)
    for g, (lo, hi) in enumerate(wins):
        a = np.searchsorted(dst_s, lo, 'left')
        b = np.searchsorted(dst_s, hi, 'left')
        sw, dw = src_s[a:b], dst_s[a:b] - lo
        per_s = []
        for s in range(2):
            ms = (sw < SPLIT) if s == 0 else (sw >= SPLIT)
            per_s.append((sw[ms], dw[ms]))
        wlists.append(per_s)
        wcount[g] = b - a

    # balance: rank windows by edge count, rank r -> (slot r//n_cores,
    # core r%n_cores).  Slots then hold near-equal counts across cores,
    # so the SPMD-uniform per-slot max is tight.
    rank = np.argsort(-wcount, kind='stable')
    assign = [[None] * NW for _ in range(n_cores)]   # assign[c][slot] = window
    for r, g in enumerate(rank):
        assign[r % n_cores][r // n_cores] = int(g)

    # ragged schedule: per (slot, s), slot count = max edges over cores
    def wn(c, w, s):
        return len(wlists[assign[c][w]][s][0])
    cnt = [[max(1, max(wn(c, w, s) for c in range(n_cores)))
            for s in range(2)] for w in range(NW)]
    t_ws = [[(cnt[w][s] + 127) // 128 for s in range(2)] for w in range(NW)]
    S_ws = [[t_ws[w][s] * 128 for s in range(2)] for w in range(NW)]
    off = [[0, 0] for _ in range(NW)]   # slot offsets into flat idx/oh arrays
    acc = 0
    for w in range(NW):
        for s in range(2):
            off[w][s] = acc
            acc += S_ws[w][s]
    TOT = acc                            # total slots
    T0M = max(t_ws[w][0] for w in range(NW))
    T1M = max(t_ws[w][1] for w in range(NW))

    cfg = dict(N=N, n_cores=n_cores, NPC=NPC, NW=NW, SPLIT=SPLIT,
               cnt=cnt, t_ws=t_ws, S_ws=S_ws, off=off, TOT=TOT,
               T0M=T0M, T1M=T1M)

    xT = np.ascontiguousarray(x.T).astype(BF16)            # [128, N]
    att_flatT = np.zeros((C_OUT_TOT, 2 * HEADS), np.float32)
    for h in range(HEADS):
        att_flatT[h * HC:(h + 1) * HC, h] = np.asarray(att_src)[h]
        att_flatT[h * HC:(h + 1) * HC, HEADS + h] = np.asarray(att_dst)[h]
    W32 = np.asarray(W, np.float32)
    watt = W32 @ att_flatT                                  # [128, 8]
    Wcat = np.concatenate([W32, watt], axis=1).astype(BF16)  # [128, 264]
    bias_bc = np.broadcast_to(np.asarray(bias, np.float32), (128, C_OUT_TOT)).copy()

    in_maps = []
    for c in range(n_cores):
        idx16 = np.zeros((128, TOT // 16), np.int16)
        ohT = np.zeros((128, TOT), FP8)   # [e_lane, slot -> local dst one-hot]
        ohF = np.zeros((128, TOT), FP8)   # [local dst lane, slot -> e one-hot]
        xT_own = np.zeros((128, NW * 128), BF16)
        for w in range(NW):
            g = assign[c][w]
            lo, hi = wins[g]
            xT_own[:, w * 128:w * 128 + (hi - lo)] = xT[:, lo:hi]
            for s in range(2):
                ssw, sdw = wlists[g][s]
                n = len(ssw)
                S = S_ws[w][s]
                cw = cnt[w][s]
                o = off[w][s]
                # [0,n) real, [n,cnt) index-0 pad (valid, zero one-hot),
                # [cnt,S) -1 (ucode trims; num_idxs_reg == cnt everywhere)
                idx = np.full(S, -1, np.int16)
                idx[:n] = (ssw - (SPLIT if s else 0)).astype(np.int16)
                idx[n:cw] = 0
                wrapped = idx.reshape(S // 16, 16).T
                idx16[:, o // 16:(o + S) // 16] = np.tile(wrapped, (8, 1))
                e_pos = np.arange(n)
                lanes = e_pos % 128
                tiles = e_pos // 128
                ohT[lanes, o + tiles * 128 + sdw] = 1.0
                ohF[sdw, o + tiles * 128 + lanes] = 1.0
        in_maps.append({
            "xT": xT, "xT_own": xT_own,
            "Wcat": Wcat, "bias_bc": bias_bc,
            "idx16": idx16, "ohT": ohT, "ohF": ohF,
        })
    return cfg, in_maps, (assign, wins)


def build_program(cfg):
    N, NPC, NW, SPLIT = (cfg[k] for k in ("N", "NPC", "NW", "SPLIT"))
    n_cores = cfg["n_cores"]
    cnt, t_ws, S_ws, off, TOT = (cfg[k] for k in ("cnt", "t_ws", "S_ws", "off", "TOT"))
    T0M, T1M = cfg["T0M"], cfg["T1M"]
    TTM = max(t_ws[w][0] + t_ws[w][1] for w in range(NW))
    dt = mybir.dt

    nc = bacc.Bacc("TRN2", target_bir_lowering=False, debug=False,
                   num_devices=n_cores, num_swdge_queues=4)
    t_xT = nc.dram_tensor("xT", (128, N), dt.bfloat16, kind="ExternalInput")
    t_xT_own = nc.dram_tensor("xT_own", (128, NW * 128), dt.bfloat16,
                              kind="ExternalInput")
    t_Wcat = nc.dram_tensor("Wcat", (C_IN, 264), dt.bfloat16, kind="ExternalInput")
    t_bias = nc.dram_tensor("bias_bc", (128, C_OUT_TOT), dt.float32, kind="ExternalInput")
    t_idx = nc.dram_tensor("idx16", (128, TOT // 16), dt.int16, kind="ExternalInput")
    t_ohT = nc.dram_tensor("ohT", (128, TOT), dt.float8e4, kind="ExternalInput")
    t_ohF = nc.dram_tensor("ohF", (128, TOT), dt.float8e4, kind="ExternalInput")
    t_htab = nc.dram_tensor("htab", (N, ROW), dt.bfloat16, kind="Internal")
    # one full 128-row block per window slot; host drops pad rows of short
    # windows when reassembling
    t_out = nc.dram_tensor("out", (NW * 128, C_OUT_TOT), dt.float32,
                           kind="ExternalOutput")

    with tile.TileContext(nc) as tc:
        with tc.tile_pool(name="const", bufs=1) as cpool, \
             tc.tile_pool(name="p2g", bufs=3) as p2g:
            Wcat_sb = cpool.tile([C_IN, 264], dt.bfloat16)
            nc.sync.dma_start(out=Wcat_sb, in_=t_Wcat.ap())
            bias_sb = cpool.tile([128, C_OUT_TOT], dt.float32)
            nc.sync.dma_start(out=bias_sb, in_=t_bias.ap())
            idx_sb = cpool.tile([128, TOT // 16], dt.int16)
            nc.sync.dma_start(out=idx_sb, in_=t_idx.ap())
            adst_sb = cpool.tile([128, NW, HEADS], dt.bfloat16)
            nc.vector.memset(adst_sb, 0)
            # Pre-zero every physical gather buffer up front (vector queue is
            # empty here): trailing -1 tiles are never written by the DMA and
            # stale NaN/Inf bit patterns would poison 0*NaN in the matmuls.
            GB_BUFS = {0: PREF + 2, 1: 4}
            GB_TM = {0: T0M, 1: T1M}
            for s in range(2):
                for zi in range(GB_BUFS[s]):
                    gz = p2g.tile([128, GB_TM[s], ROW], dt.bfloat16,
                                  tag=f"gb{s}", bufs=GB_BUFS[s],
                                  name=f"gbz{s}_{zi}")
                    nc.vector.memset(gz, 0)

            # ---------- phase 1b: a_dst for assigned windows ----------
            # independent of the table; runs first so adst is ready long
            # before phase-2 scores.  One batched load, then matmul slices.
            with tc.tile_pool(name="p1bx", bufs=1) as p1bx, \
                 tc.tile_pool(name="p1bps", bufs=4, space="PSUM") as p1bps:
                xo = p1bx.tile([128, NW * 128], dt.bfloat16)
                nc.scalar.dma_start(out=xo, in_=t_xT_own.ap())
                for w in range(NW):
                    ps_l2 = p1bps.tile([128, 2 * HEADS], dt.float32, tag="ps_l2")
                    nc.tensor.matmul(out=ps_l2,
                                     lhsT=xo[:, w * 128:(w + 1) * 128],
                                     rhs=Wcat_sb[:, C_OUT_TOT:264],
                                     start=True, stop=True)
                    nc.scalar.copy(out=adst_sb[:, w, :],
                                   in_=ps_l2[:, HEADS:2 * HEADS])

            # ---------- phase 1: h table ----------
            # 4 node-tiles batched per table write (fewer serial sync-queue
            # dispatches); PSUM->SBUF copies alternate scalar/vector engines.
            # lo/hi fences (RAW through DRAM, untracked by Tile) are emitted
            # inline so the lo fence clears at ~50% of phase 1 and lo-half
            # gathers start while the hi half is still being built.
            htab_w_lo, htab_w_hi = [], []
            fence_lo = fence_hi = None
            GRP = 8
            CHUNK = 8192  # 64 node-tiles per chunk (16 KB/partition bf16)
            with tc.tile_pool(name="p1x", bufs=2) as p1x, \
                 tc.tile_pool(name="p1h", bufs=4) as p1h, \
                 tc.tile_pool(name="p1ps", bufs=6, space="PSUM") as p1ps:
                # pre-zero the row pads once per physical buffer; the per-tile
                # copy only writes cols 0:264.
                for zi in range(4):
                    hz = p1h.tile([128, GRP, ROW], dt.bfloat16, tag="hsb",
                                  name=f"hz{zi}")
                    nc.vector.memset(hz[:, :, 264:ROW], 0)
                for ci in range(0, N, CHUNK):
                    cw = min(CHUNK, N - ci)
                    xc = p1x.tile([128, CHUNK], dt.bfloat16, tag="xc")
                    nc.sync.dma_start(out=xc[:, 0:cw], in_=t_xT.ap()[:, ci:ci + cw])
                    for g0 in range(0, cw, 128 * GRP):
                        gw = min(128 * GRP, cw - g0)
                        ng = (gw + 127) // 128
                        hsb = p1h.tile([128, GRP, ROW], dt.bfloat16, tag="hsb")
                        for gi in range(ng):
                            nt0 = g0 + gi * 128
                            nn = min(128, cw - nt0)
                            ps_h = p1ps.tile([128, 264], dt.float32, tag="ps_h")
                            nc.tensor.matmul(out=ps_h[0:nn, :],
                                             lhsT=xc[:, nt0:nt0 + nn],
                                             rhs=Wcat_sb, start=True, stop=True)
                            if gi % 2 == 0:
                                nc.scalar.copy(out=hsb[0:nn, gi, 0:264],
                                               in_=ps_h[0:nn, :])
                            else:
                                nc.vector.tensor_copy(out=hsb[0:nn, gi, 0:264],
                                                      in_=ps_h[0:nn, :])
                        n0 = ci + g0
                        nrow = gw
                        ngf = nrow // 128
                        wrs = []
                        if ngf:
                            wrs.append(nc.sync.dma_start(
                                out=t_htab.ap()[n0:n0 + ngf * 128, :].rearrange(
                                    "(g q) f -> q g f", g=ngf),
                                in_=hsb[:, 0:ngf, :]))
                        rem = nrow - ngf * 128
                        if rem:
                            wrs.append(nc.sync.dma_start(
                                out=t_htab.ap()[n0 + ngf * 128:n0 + nrow, :],
                                in_=hsb[0:rem, ngf, :]))
                        for wr in wrs:
                            if n0 < SPLIT:
                                htab_w_lo.append(wr)
                            if n0 + nrow > SPLIT:
                                htab_w_hi.append(wr)
                        if fence_lo is None and n0 + nrow >= SPLIT:
                            fence_lo = nc.sync.nop(hint="htab_fence_lo", nofuse=True)
                            for _wi in htab_w_lo:
                                _br.add_dep_helper(fence_lo.ins, _wi.ins,
                                                   reason="htab lo RAW")
                fence_hi = nc.sync.nop(hint="htab_fence_hi", nofuse=True)
                for _wi in htab_w_hi:
                    _br.add_dep_helper(fence_hi.ins, _wi.ins, reason="htab hi RAW")

            # ---------- phase 2 ----------
            ap_lo = t_htab.ap()[0:SPLIT, :]
            ap_hi = t_htab.ap()[SPLIT:N, :]
            with tc.tile_pool(name="p2o", bufs=4) as p2o, \
                 tc.tile_pool(name="p2m", bufs=3) as p2m, \
                 tc.tile_pool(name="p2s", bufs=3) as p2s, \
                 tc.tile_pool(name="p2ps", bufs=4, space="PSUM") as p2ps, \
                 tc.tile_pool(name="p2pse", bufs=2, space="PSUM") as p2pse:
                def emit_gather(w, s):
                    g = p2g.tile([128, t_ws[w][s], ROW], dt.bfloat16,
                                 tag=f"gb{s}", bufs=GB_BUFS[s],
                                 padded_shape=[128, GB_TM[s], ROW],
                                 name=f"gb{s}_{w}")
                    _g = nc.gpsimd.dma_gather(
                        out_ap=g, in_ap=(ap_lo if s == 0 else ap_hi),
                        idxs_ap=idx_sb[:, off[w][s] // 16:(off[w][s] + S_ws[w][s]) // 16],
                        num_idxs=S_ws[w][s], num_idxs_reg=cnt[w][s],
                        elem_size=ROW,
                        single_packet=False, queue_num=(w * 2 + s) % 4,
                    )
                    _br.add_dep_helper(_g.ins,
                                       (fence_lo if s == 0 else fence_hi).ins,
                                       reason="htab RAW fence")
                    return g

                pend_norm = []

                def _norm_flush(w, ps_win):
                    rcp = p2s.tile([128, HEADS], dt.float32, tag="rcp",
                                   name=f"rcp{w}")
                    nc.vector.reciprocal(out=rcp, in_=ps_win[:, 256:260])
                    osb = p2s.tile([128, C_OUT_TOT], dt.float32, tag="osb",
                                   name=f"osb{w}")
                    for h in range(HEADS):
                        nc.scalar.mul(out=osb[:, h * HC:(h + 1) * HC],
                                      in_=ps_win[:, h * HC:(h + 1) * HC],
                                      mul=rcp[:, h:h + 1])
                    nc.vector.tensor_tensor(out=osb, in0=osb, in1=bias_sb,
                                            op=mybir.AluOpType.add)
                    nc.sync.dma_start(out=t_out.ap()[w * 128:(w + 1) * 128, :],
                                      in_=osb)

                pend = {}
                for w in range(min(PREF, NW)):
                    pend[w] = emit_gather(w, 0)

                for w in range(NW):
                    t0, t1 = t_ws[w]
                    tt = t0 + t1
                    o0 = off[w][0]
                    SW = S_ws[w][0] + S_ws[w][1]   # contiguous (w,0),(w,1)
                    gb = [pend.pop(w), emit_gather(w, 1)]
                    if w + PREF < NW:
                        pend[w + PREF] = emit_gather(w + PREF, 0)
                    ohT_b = p2o.tile([128, SW], dt.float8e4, tag="ohT",
                                     padded_shape=[128, TTM * 128])
                    ohF_b = p2o.tile([128, SW], dt.float8e4, tag="ohF",
                                     padded_shape=[128, TTM * 128])
                    nc.sync.dma_start(out=ohT_b, in_=t_ohT.ap()[:, o0:o0 + SW])
                    nc.sync.dma_start(out=ohF_b, in_=t_ohF.ap()[:, o0:o0 + SW])

                    # a_dst per edge: [128e, 4] per tile -> ps_adst[:, t, :]
                    ps_adst = p2pse.tile([128, tt, HEADS], dt.float32, tag="ps_adst",
                                         padded_shape=[128, TTM, HEADS])
                    for t in range(tt):
                        nc.tensor.matmul(
                            out=ps_adst[:, t, :],
                            lhsT=ohF_b[:, t * 128:(t + 1) * 128],
                            rhs=adst_sb[:, w, :],
                            start=True, stop=True)

                    # e = exp(lrelu(a_src + a_dst)) -> msg[:, :, 256:260]
                    msg = p2m.tile([128, tt, 264], dt.bfloat16, tag="msg",
                                   padded_shape=[128, TTM, 264])
                    e_tmp = p2s.tile([128, tt, HEADS], dt.float32, tag="e_tmp",
                                     padded_shape=[128, TTM, HEADS])
                    nc.vector.tensor_tensor(
                        out=e_tmp[:, 0:t0, :], in0=ps_adst[:, 0:t0, :],
                        in1=gb[0][:, :, C_OUT_TOT:C_OUT_TOT + HEADS],
                        op=mybir.AluOpType.add)
                    nc.vector.tensor_tensor(
                        out=e_tmp[:, t0:tt, :], in0=ps_adst[:, t0:tt, :],
                        in1=gb[1][:, :, C_OUT_TOT:C_OUT_TOT + HEADS],
                        op=mybir.AluOpType.add)
                    e2 = p2s.tile([128, tt, HEADS], dt.float32, tag="e2",
                                  padded_shape=[128, TTM, HEADS])
                    nc.scalar.activation(out=e2, in_=e_tmp,
                                         func=mybir.ActivationFunctionType.Prelu,
                                         alpha=NEG_SLOPE)
                    nc.scalar.activation(out=msg[:, :, 256:260], in_=e2,
                                         func=mybir.ActivationFunctionType.Exp)

                    # msg = h * ex (broadcast per head), batched per stream
                    for s, lo_t, hi_t in ((0, 0, t0), (1, t0, tt)):
                        exb = msg[:, lo_t:hi_t, 256:260].unsqueeze(3).broadcast_to(
                            [128, hi_t - lo_t, HEADS, HC])
                        nc.vector.tensor_tensor(
                            out=msg[:, lo_t:hi_t, 0:C_OUT_TOT].rearrange(
                                "p t (h c) -> p t h c", h=HEADS),
                            in0=gb[s][:, :, 0:C_OUT_TOT].rearrange(
                                "p t (h c) -> p t h c", h=HEADS),
                            in1=exb, op=mybir.AluOpType.mult)

                    # aggregate: psum[p, 0:256] += msg, psum[p, 256:260] += ex
                    ps_win = p2ps.tile([128, 260], dt.float32, tag="ps_win")
                    for t in range(tt):
                        nc.tensor.matmul(
                            out=ps_win,
                            lhsT=ohT_b[:, t * 128:(t + 1) * 128],
                            rhs=msg[:, t, 0:260],
                            start=(t == 0), stop=(t == tt - 1))

                    # normalize + bias, deferred 2 windows: emitting it
                    # here would park scalar/vector queue entries whose deps
                    # (this window's aggregation) are not met yet, stalling
                    # the NEXT windows' score ops behind them in queue order.
                    pend_norm.append((w, ps_win))
                    if len(pend_norm) > 2:
                        _norm_flush(*pend_norm.pop(0))

                for args in pend_norm:
                    _norm_flush(*args)

    nc.finalize()
    return nc


def register_ntff_hook():
    import types
    import antenv
    if getattr(antenv, 'axon_hooks', None) is not None:
        return
    mod = types.ModuleType('antenv.axon_hooks')
    _hook = [None]
    mod.set_axon_ntff_profile_hook = lambda h: _hook.__setitem__(0, h)
    mod.get_axon_ntff_profile_hook = lambda: _hook[0]
    sys.modules['antenv.axon_hooks'] = mod
    antenv.axon_hooks = mod
    try:
        from trn_agent_boot.trn_boot import _ntff_profile_via_ctypes
        mod.set_axon_ntff_profile_hook(
            _ntff_profile_via_ctypes('/opt/axon/libaxon_pjrt.so'))
    except Exception:
        pass


def run(x, edge_index, W, att_src, att_dst, bias, n_cores=8, trace=False):
    cfg, in_maps, (assign, wins) = host_prep(x, edge_index, W, att_src, att_dst,
                                             bias, n_cores)
    nc = build_program(cfg)
    if trace:
        register_ntff_hook()
    r = bass_utils.run_bass_kernel_spmd(nc, in_maps,
                                        core_ids=list(range(n_cores)),
                                        trace=trace)
    N = cfg["N"]
    out = np.empty((N, C_OUT_TOT), np.float32)
    for c in range(n_cores):
        oc = r.results[c]["out"]
        for w in range(cfg["NW"]):
            lo, hi = wins[assign[c][w]]
            out[lo:hi] = oc[w * 128:w * 128 + (hi - lo)]
    return out, r


# ----------------------------------------------------------------------------
# Self-contained harness entry point: full inputs in, full output out.
# ----------------------------------------------------------------------------
import os as _os


def kernel(x, edge_index, W, att_src, att_dst, bias):
    x = np.asarray(x, np.float32)
    edge_index = np.asarray(edge_index)
    W = np.asarray(W, np.float32)
    att_src = np.asarray(att_src, np.float32)
    att_dst = np.asarray(att_dst, np.float32)
    bias = np.asarray(bias, np.float32)
    trace = _os.environ.get("GAT_TRACE", "0") == "1"
    out, r = run(x, edge_index, W, att_src, att_dst, bias, n_cores=8, trace=trace)
    if trace and r.exec_time_ns is not None:
        print(f"HW exec time: {r.exec_time_ns} ns")
    return np.ascontiguousarray(out.astype(np.float32))
